# revision 4
# baseline (speedup 1.0000x reference)
"""Trainium2 Bass kernel for entity-attention input scaling.

Computes, per batch row b:
    A_k = wd[b] @ e_k[b]          (k = 1, 2)   [S]
    alpha_k = softmax(A_k)
    out[b]  = wM[b] * 0.5 * (alpha_1^2 + alpha_2^2)[:, None]

Sharding: pure data parallel over the batch dim. B=32 batches are split
4-per-core over 8 NeuronCores; no cross-core communication.

Per-core pipeline (per local batch):
  - wd streamed in 2MB contiguous chunks -> SBUF [128, 4096]
    (s = 2048*c + 16*p + j; p = partition, j in 0..15)
  - each [128s, 128d] block PE-transposed (via identity matmul) -> PSUM,
    copied to SBUF (alternating DVE/ACT), used as the stationary operand of
    an fp32 matmul against ehat = 2*[e1, e2] [128d, 2]; accumulated over the
    two d-blocks into one PSUM bank psA[128, 64] holding 2*A_k per (tile, k).
  - softmax stats: row max via DVE reduce + PE transpose + DVE reduce;
    exp with accumulate on ACT (per-partition sums), cross-partition sum via
    ones-vector matmul; ln Z on ACT.  The full normalizer is folded into the
    exponent: 0.5*alpha_k^2 = exp(2A_k - m2 - 2 ln Z'_k + ln 0.5), added to
    psA with a rank-1 matmul, so a single exp yields the alpha contributions.
  - out = wM * alpha via per-partition tensor_scalar (split DVE/ACT), stored
    with 2MB contiguous DMAs.
"""

import numpy as np
from contextlib import ExitStack

import concourse.bass as bass
import concourse.bacc as bacc
import concourse.tile as tile
from concourse import mybir
from concourse.bass_utils import run_bass_kernel_spmd

B, S, D = 32, 4096, 256
N_CORES = 8
BPC = B // N_CORES          # batches per core
CHUNK = 2048                # S-rows per DMA chunk (2MB)
NCH = S // CHUNK            # chunks per batch
JP = CHUNK // 128           # 128-row tiles per chunk
NT = S // 128               # 128-row tiles per batch
F32 = mybir.dt.float32
AF = mybir.ActivationFunctionType
ALU = mybir.AluOpType
LN2 = float(np.log(2.0))
CORE_IDS = list(range(N_CORES))

_cache: dict = {}


def _build():
    nc = bacc.Bacc("TRN2", target_bir_lowering=False, debug=False,
                   num_devices=N_CORES)
    wd_h = nc.declare_dram_parameter("wd", [BPC, S, D], F32, isOutput=False)
    wM_h = nc.declare_dram_parameter("wM", [BPC, S, D], F32, isOutput=False)
    # ehat[d_in_blk, (b*2+blk)*2 + k] = 2 * e_k[b, blk*128 + d_in_blk]
    eh_h = nc.declare_dram_parameter("ehat", [128, BPC * 4], F32, isOutput=False)
    id_h = nc.declare_dram_parameter("ident", [128, 128], F32, isOutput=False)
    oc_h = nc.declare_dram_parameter("onescol", [128, 1], F32, isOutput=False)
    nr_h = nc.declare_dram_parameter("neghalfrow", [1, 128], F32, isOutput=False)
    out_h = nc.declare_dram_parameter("out", [BPC, S, D], F32, isOutput=True)

    def chunk_view(h, b, c):
        # [CHUNK, D] contiguous rows -> [128, JP*D]; s = CHUNK*c + JP*p + j
        return h[b, CHUNK * c:CHUNK * (c + 1), :].rearrange(
            "(p j) d -> p (j d)", p=128)

    with tile.TileContext(nc) as tc, ExitStack() as ctx:
        consts = ctx.enter_context(tc.tile_pool(name="consts", bufs=1))
        wd_pool = ctx.enter_context(tc.tile_pool(name="wdp", bufs=3))
        wm_pool = ctx.enter_context(tc.tile_pool(name="wmp", bufs=3))
        out_pool = ctx.enter_context(tc.tile_pool(name="outp", bufs=2))
        wdt_pool = ctx.enter_context(tc.tile_pool(name="wdtp", bufs=3))
        sm_pool = ctx.enter_context(tc.tile_pool(name="smalls", bufs=2))
        al_pool = ctx.enter_context(tc.tile_pool(name="alphas", bufs=2))
        pst_pool = ctx.enter_context(tc.tile_pool(name="pst", bufs=3, space="PSUM"))
        psA_pool = ctx.enter_context(tc.tile_pool(name="psA", bufs=2, space="PSUM"))
        pss_pool = ctx.enter_context(tc.tile_pool(name="pss", bufs=2, space="PSUM"))

        ident = consts.tile([128, 128], F32)
        nc.sync.dma_start(ident[:], id_h[:])
        onescol = consts.tile([128, 1], F32)
        nc.sync.dma_start(onescol[:], oc_h[:])
        neghalf = consts.tile([1, 128], F32)
        nc.sync.dma_start(neghalf[:], nr_h[:])
        ehat = consts.tile([128, BPC * 4], F32)
        nc.sync.dma_start(ehat[:], eh_h[:])

        for b in range(BPC):
            # ---- phase 1: logits 2*A into psA[128, 2*NT] ----
            psA = psA_pool.tile([128, 2 * NT], F32)
            ncopy = 0
            for c in range(NCH):
                wd_ch = wd_pool.tile([128, JP * D], F32, tag="wd")
                nc.sync.dma_start(wd_ch[:], chunk_view(wd_h, b, c))
                for jj in range(0, JP, 2):  # two j-tiles (4 blocks) per psum tile
                    ps_t = pst_pool.tile([128, 512], F32, tag="pst")
                    for q in range(4):
                        j = jj + q // 2
                        blk = q % 2
                        off = j * D + blk * 128
                        # start=True zeroes the WHOLE 2KB psum bank: only the
                        # first write into this tile may set it.
                        nc.tensor.matmul(
                            ps_t[:, q * 128:(q + 1) * 128],
                            wd_ch[:, off:off + 128],
                            ident[:],
                            is_transpose=True,
                            start=(q == 0), stop=(q == 3),
                            skip_group_check=True)
                    wdT = wdt_pool.tile([128, 512], F32, tag="wdT")
                    eng = nc.vector if (ncopy % 2 == 0) else nc.scalar
                    if eng is nc.vector:
                        eng.tensor_copy(wdT[:], ps_t[:])
                    else:
                        eng.copy(wdT[:], ps_t[:])
                    ncopy += 1
                    for q in range(4):
                        j = jj + q // 2
                        blk = q % 2
                        t = c * JP + j
                        # whole-bank zero on start: only the first matmul into
                        # psA sets start=True; later tiles rely on the bank
                        # already being marked (first touch replaces).
                        nc.tensor.matmul(
                            psA[:, 2 * t:2 * t + 2],
                            wdT[:, q * 128:(q + 1) * 128],
                            ehat[:, (b * 2 + blk) * 2:(b * 2 + blk) * 2 + 2],
                            start=(c == 0 and jj == 0 and q == 0),
                            stop=False,
                            skip_group_check=True)

            # ---- phase 2: softmax stats ----
            mx = sm_pool.tile([128, 1], F32, tag="mx")
            nc.vector.tensor_reduce(mx[:], psA[:], axis=mybir.AxisListType.X,
                                    op=ALU.max)
            tmax = pss_pool.tile([1, 128], F32, tag="pssm")
            nc.tensor.transpose(tmax[:], mx[:], ident[:])
            m2 = sm_pool.tile([1, 1], F32, tag="m2")
            nc.vector.tensor_reduce(m2[:], tmax[:], axis=mybir.AxisListType.X,
                                    op=ALU.max)
            # broadcast -0.5*m2 to all partitions: [128,1] psum
            mneg_ps = pss_pool.tile([128, 1], F32, tag="pssm")
            nc.tensor.matmul(mneg_ps[:], neghalf[:], m2[:], start=True, stop=True)
            mneg = sm_pool.tile([128, 1], F32, tag="mneg")
            nc.scalar.copy(mneg[:], mneg_ps[:])

            # exp(A - maxA) per k with per-partition accumulate
            psA_v = psA[:].rearrange("p (t k) -> p k t", k=2)
            s12 = sm_pool.tile([128, 2], F32, tag="s12")
            for k in range(2):
                escr = sm_pool.tile([128, NT], F32, tag="escr")
                nc.scalar.activation(escr[:], psA_v[:, k, :], AF.Exp,
                                     bias=mneg[:], scale=0.5,
                                     accum_out=s12[:, k:k + 1])
            # cross-partition sum -> Z'[1,2]
            zsum = pss_pool.tile([1, 2], F32, tag="pssm")
            nc.tensor.matmul(zsum[:], onescol[:], s12[:], start=True, stop=True)
            lnz = sm_pool.tile([1, 2], F32, tag="lnz")
            nc.scalar.activation(lnz[:], zsum[:], AF.Ln)
            # y = 4*lnZ' + 2*m2 + 2*ln2  (= -2*beta); psA += (-0.5)*y
            negb = sm_pool.tile([1, 2], F32, tag="negb")
            nc.vector.tensor_scalar(negb[:], lnz[:], 2.0, None, op0=ALU.mult)
            nc.vector.tensor_scalar(negb[:], negb[:], m2[:], 2.0,
                                    op0=ALU.add, op1=ALU.mult)
            nc.vector.tensor_scalar(negb[:], negb[:], 2.0 * LN2, None,
                                    op0=ALU.add)
            negb_rep = sm_pool.tile([1, 2 * NT], F32, tag="negbr")
            nbr_v = negb_rep[:].rearrange("p (t k) -> p t k", k=2)
            nc.vector.tensor_copy(nbr_v[:], negb[:].unsqueeze(1).broadcast_to([1, NT, 2]))
            nc.tensor.matmul(psA[:], neghalf[:], negb_rep[:],
                             start=False, stop=True, skip_group_check=True)

            # alpha = sum_k exp(2A_k + beta_k) = 0.5*(a1^2 + a2^2)
            a2 = al_pool.tile([128, 2 * NT], F32, tag="a2")
            nc.scalar.activation(a2[:], psA[:], AF.Exp)
            alpha = al_pool.tile([128, NT], F32, tag="alpha")
            a2_v = a2[:].rearrange("p (t k) -> p k t", k=2)
            nc.vector.tensor_add(alpha[:], a2_v[:, 0, :], a2_v[:, 1, :])

            # ---- phase 3: out = wM * alpha ----
            for c in range(NCH):
                wm_ch = wm_pool.tile([128, JP * D], F32, tag="wm")
                nc.sync.dma_start(wm_ch[:], chunk_view(wM_h, b, c))
                out_ch = out_pool.tile([128, JP * D], F32, tag="out")
                for j in range(JP):
                    t = c * JP + j
                    sl = slice(j * D, (j + 1) * D)
                    if t % 4 == 3:
                        nc.scalar.mul(out_ch[:, sl], wm_ch[:, sl],
                                      alpha[:, t:t + 1])
                    else:
                        nc.vector.tensor_scalar_mul(out_ch[:, sl], wm_ch[:, sl],
                                                    alpha[:, t:t + 1])
                nc.scalar.dma_start(chunk_view(out_h, b, c), out_ch[:])

    nc.finalize()
    return nc


def _get_nc():
    if "nc" not in _cache:
        _cache["nc"] = _build()
    return _cache["nc"]


def _in_maps(wM, wd, e1, e2):
    eh = np.stack([2.0 * e1, 2.0 * e2], axis=-1)  # [B, D, 2]
    ident = np.eye(128, dtype=np.float32)
    onescol = np.ones((128, 1), dtype=np.float32)
    neghalf = np.full((1, 128), -0.5, dtype=np.float32)
    maps = []
    for i in range(N_CORES):
        sl = slice(i * BPC, (i + 1) * BPC)
        ehc = (eh[sl].reshape(BPC, 2, 128, 2).transpose(2, 0, 1, 3)
               .reshape(128, BPC * 4).copy())
        maps.append({
            "wd": np.ascontiguousarray(wd[sl]),
            "wM": np.ascontiguousarray(wM[sl]),
            "ehat": ehc,
            "ident": ident,
            "onescol": onescol,
            "neghalfrow": neghalf,
        })
    return maps


def _run(wM, wd, e1, e2, **kw):
    wM = np.asarray(wM, dtype=np.float32)
    wd = np.asarray(wd, dtype=np.float32)
    e1 = np.asarray(e1, dtype=np.float32)
    e2 = np.asarray(e2, dtype=np.float32)
    nc = _get_nc()
    res = run_bass_kernel_spmd(nc, _in_maps(wM, wd, e1, e2), CORE_IDS, **kw)
    out = np.concatenate([r["out"] for r in res.results], axis=0)
    return out, res


def kernel(wM, wd, e1, e2):
    out, _ = _run(wM, wd, e1, e2)
    return out


# revision 11
# speedup vs baseline: 1.3556x; 1.3556x over previous
"""Trainium2 Bass kernel for entity-attention input scaling.

Computes, per batch row b:
    A_k = wd[b] @ e_k[b]          (k = 1, 2)   [S]
    alpha_k = softmax(A_k)
    out[b]  = wM[b] * 0.5 * (alpha_1^2 + alpha_2^2)[:, None]

Sharding: pure data parallel over the batch dim. B=32 batches are split
4-per-core over 8 NeuronCores; no cross-core communication.

Per-core pipeline (per local batch), memory-roofline bound (~50MB DMA/core):
  - wd streamed in 2MB contiguous chunks -> SBUF [128, 4096]
    (s = 2048*c + 16*p + j; p = partition, j in 0..15)
  - logits on the DVE: one fused tensor_tensor_reduce per [128, 256] tile
    against host-broadcast e_k -> psA[128, 64] (= 2*A_k per (tile, k) col).
    No PE transposes / stationary loads (fp32 LDWEIGHTS is 2-pass and was
    the bottleneck in the matmul formulation).
  - softmax stats: row max via DVE reduce + PE transpose + DVE reduce;
    exp with accumulate on ACT (per-partition sums), cross-partition sum via
    ones-vector matmul; ln Z on ACT.  The full normalizer is folded into the
    exponent: 0.5*alpha_k^2 = exp(2A_k - m2 - 2 ln Z'_k + ln 0.5); the
    per-column constant is broadcast to [128, 64] with a rank-1 matmul and
    added on the DVE, so a single exp yields the alpha contributions.
  - out = wM * alpha via per-partition scaled-copy on ACT, stored with 2MB
    contiguous DMAs (loads on the SP DGE ring, stores on the ACT ring).
"""

import numpy as np
from contextlib import ExitStack

import concourse.bass as bass
import concourse.bacc as bacc
import concourse.tile as tile
from concourse import mybir
from concourse.bass_utils import run_bass_kernel_spmd

B, S, D = 32, 4096, 256
N_CORES = 8
BPC = B // N_CORES          # batches per core
CHUNK = 2048                # S-rows per DMA chunk (2MB)
NCH = S // CHUNK            # chunks per batch
JP = CHUNK // 128           # 128-row tiles per chunk
NT = S // 128               # 128-row tiles per batch
F32 = mybir.dt.float32
AF = mybir.ActivationFunctionType
ALU = mybir.AluOpType
LN2 = float(np.log(2.0))
CORE_IDS = list(range(N_CORES))

_cache: dict = {}


def _build():
    nc = bacc.Bacc("TRN2", target_bir_lowering=False, debug=False,
                   num_devices=N_CORES)
    wd_h = nc.declare_dram_parameter("wd", [BPC, S, D], F32, isOutput=False)
    wM_h = nc.declare_dram_parameter("wM", [BPC, S, D], F32, isOutput=False)
    # ebc[p, ((b*2 + k)*D + d)] = e_k[b, d]  (same for every partition p)
    eb_h = nc.declare_dram_parameter("ebc", [128, BPC * 2 * D], F32,
                                     isOutput=False)
    id_h = nc.declare_dram_parameter("ident", [128, 128], F32, isOutput=False)
    oc_h = nc.declare_dram_parameter("onescol", [128, 1], F32, isOutput=False)
    nr_h = nc.declare_dram_parameter("negonerow", [1, 128], F32, isOutput=False)
    or_h = nc.declare_dram_parameter("onesrow", [1, 128], F32, isOutput=False)
    out_h = nc.declare_dram_parameter("out", [BPC, S, D], F32, isOutput=True)

    def chunk_view(h, b, c):
        # [CHUNK, D] contiguous rows -> [128, JP*D]; s = CHUNK*c + JP*p + j
        return h[b, CHUNK * c:CHUNK * (c + 1), :].rearrange(
            "(p j) d -> p (j d)", p=128)

    with tile.TileContext(nc) as tc, ExitStack() as ctx:
        consts = ctx.enter_context(tc.tile_pool(name="consts", bufs=1))
        wd_pool = ctx.enter_context(tc.tile_pool(name="wdp", bufs=3))
        wm_pool = ctx.enter_context(tc.tile_pool(name="wmp", bufs=3))
        out_pool = ctx.enter_context(tc.tile_pool(name="outp", bufs=2))
        scr_pool = ctx.enter_context(tc.tile_pool(name="scrp", bufs=3))
        sm_pool = ctx.enter_context(tc.tile_pool(name="smalls", bufs=2))
        al_pool = ctx.enter_context(tc.tile_pool(name="alphas", bufs=2))
        pss_pool = ctx.enter_context(tc.tile_pool(name="pss", bufs=2, space="PSUM"))

        ident = consts.tile([128, 128], F32)
        nc.sync.dma_start(ident[:], id_h[:])
        onescol = consts.tile([128, 1], F32)
        nc.sync.dma_start(onescol[:], oc_h[:])
        negone = consts.tile([1, 128], F32)
        nc.sync.dma_start(negone[:], nr_h[:])
        onesrow = consts.tile([1, 128], F32)
        nc.sync.dma_start(onesrow[:], or_h[:])
        ebc = consts.tile([128, BPC * 2 * D], F32)
        nc.sync.dma_start(ebc[:], eb_h[:])

        for b in range(BPC):
            # ---- phase 1: logits 2*A_k into psA[128, 2*NT] (SBUF) ----
            psA = al_pool.tile([128, 2 * NT], F32, tag="psA")
            for c in range(NCH):
                wd_ch = wd_pool.tile([128, JP * D], F32, tag="wd")
                nc.sync.dma_start(wd_ch[:], chunk_view(wd_h, b, c))
                for j in range(JP):
                    t = c * JP + j
                    wsl = wd_ch[:, j * D:(j + 1) * D]
                    for k in range(2):
                        scr = scr_pool.tile([128, D], F32, tag="scr")
                        nc.vector.scalar_tensor_tensor(
                            scr[:], wsl, 1.0,
                            ebc[:, (b * 2 + k) * D:(b * 2 + k + 1) * D],
                            op0=ALU.mult, op1=ALU.mult,
                            accum_out=psA[:, 2 * t + k:2 * t + k + 1])

            # ---- phase 2: softmax ----
            mx = sm_pool.tile([128, 1], F32, tag="mx")
            nc.vector.tensor_reduce(mx[:], psA[:], axis=mybir.AxisListType.X,
                                    op=ALU.max)
            tmax = pss_pool.tile([1, 128], F32, tag="pssm")
            nc.tensor.transpose(tmax[:], mx[:], ident[:])
            m2 = sm_pool.tile([1, 1], F32, tag="m2")
            nc.vector.tensor_reduce(m2[:], tmax[:], axis=mybir.AxisListType.X,
                                    op=ALU.max)
            # broadcast -maxA to all partitions: [128,1] psum -> sbuf
            mneg_ps = pss_pool.tile([128, 1], F32, tag="pssm")
            nc.tensor.matmul(mneg_ps[:], negone[:], m2[:], start=True, stop=True)
            mneg = sm_pool.tile([128, 1], F32, tag="mneg")
            nc.scalar.copy(mneg[:], mneg_ps[:])

            # E_k = exp(A_k - maxA) with per-partition accumulate
            psA_v = psA[:].rearrange("p (t k) -> p k t", k=2)
            E = al_pool.tile([128, 2 * NT], F32, tag="E")
            E_v = E[:].rearrange("p (t k) -> p k t", k=2)
            s12 = sm_pool.tile([128, 2], F32, tag="s12")
            for k in range(2):
                nc.scalar.activation(E_v[:, k, :], psA_v[:, k, :], AF.Exp,
                                     bias=mneg[:], scale=1.0,
                                     accum_out=s12[:, k:k + 1])
            # cross-partition sum -> Z'[1,2]; c_k = 0.5 / Z'_k^2
            zsum = pss_pool.tile([1, 2], F32, tag="pssm")
            nc.tensor.matmul(zsum[:], onescol[:], s12[:], start=True, stop=True)
            zinv = sm_pool.tile([1, 2], F32, tag="zinv")
            nc.vector.reciprocal(zinv[:], zsum[:])
            zz = sm_pool.tile([1, 2], F32, tag="zz")
            nc.vector.tensor_scalar(zz[:], zinv[:], 0.5, None, op0=ALU.mult)
            nc.vector.tensor_mul(zz[:], zz[:], zinv[:])
            # broadcast c_k to all partitions: [128, 2] psum -> sbuf
            c_ps = pss_pool.tile([128, 2], F32, tag="pssm")
            nc.tensor.matmul(c_ps[:], onesrow[:], zz[:], start=True, stop=True)
            c12 = sm_pool.tile([128, 2], F32, tag="c12")
            nc.scalar.copy(c12[:], c_ps[:])

            # alpha = c_1*E_1^2 + c_2*E_2^2
            esq = al_pool.tile([128, 2 * NT], F32, tag="esq")
            nc.vector.tensor_mul(esq[:], E[:], E[:])
            esq_v = esq[:].rearrange("p (t k) -> p k t", k=2)
            atmp = al_pool.tile([128, NT], F32, tag="atmp")
            nc.vector.tensor_scalar_mul(atmp[:], esq_v[:, 1, :], c12[:, 1:2])
            alpha = al_pool.tile([128, NT], F32, tag="alpha")
            nc.vector.scalar_tensor_tensor(alpha[:], esq_v[:, 0, :],
                                           c12[:, 0:1], atmp[:],
                                           op0=ALU.mult, op1=ALU.add)

            # ---- phase 3: out = wM * alpha ----
            for c in range(NCH):
                wm_ch = wm_pool.tile([128, JP * D], F32, tag="wm")
                nc.sync.dma_start(wm_ch[:], chunk_view(wM_h, b, c))
                out_ch = out_pool.tile([128, JP * D], F32, tag="out")
                for j in range(JP):
                    t = c * JP + j
                    sl = slice(j * D, (j + 1) * D)
                    nc.scalar.mul(out_ch[:, sl], wm_ch[:, sl],
                                  alpha[:, t:t + 1])
                nc.scalar.dma_start(chunk_view(out_h, b, c), out_ch[:])

    nc.finalize()
    return nc


def _get_nc():
    if "nc" not in _cache:
        _cache["nc"] = _build()
    return _cache["nc"]


def _in_maps(wM, wd, e1, e2):
    ident = np.eye(128, dtype=np.float32)
    onescol = np.ones((128, 1), dtype=np.float32)
    negone = np.full((1, 128), -1.0, dtype=np.float32)
    onesrow = np.ones((1, 128), dtype=np.float32)
    maps = []
    for i in range(N_CORES):
        sl = slice(i * BPC, (i + 1) * BPC)
        # [BPC, 2, D] -> row vector repeated over 128 partitions
        ebc = np.stack([e1[sl], e2[sl]], axis=1).reshape(1, BPC * 2 * D)
        ebc = np.ascontiguousarray(np.broadcast_to(ebc, (128, BPC * 2 * D)))
        maps.append({
            "wd": np.ascontiguousarray(wd[sl]),
            "wM": np.ascontiguousarray(wM[sl]),
            "ebc": ebc,
            "ident": ident,
            "onescol": onescol,
            "negonerow": negone,
            "onesrow": onesrow,
        })
    return maps


def _run(wM, wd, e1, e2, **kw):
    wM = np.asarray(wM, dtype=np.float32)
    wd = np.asarray(wd, dtype=np.float32)
    e1 = np.asarray(e1, dtype=np.float32)
    e2 = np.asarray(e2, dtype=np.float32)
    nc = _get_nc()
    res = run_bass_kernel_spmd(nc, _in_maps(wM, wd, e1, e2), CORE_IDS, **kw)
    out = np.concatenate([r["out"] for r in res.results], axis=0)
    return out, res


def kernel(wM, wd, e1, e2):
    out, _ = _run(wM, wd, e1, e2)
    return out


# revision 13
# speedup vs baseline: 1.4363x; 1.0596x over previous
"""Trainium2 Bass kernel for entity-attention input scaling.

Computes, per batch row b:
    A_k = wd[b] @ e_k[b]          (k = 1, 2)   [S]
    alpha_k = softmax(A_k)
    out[b]  = wM[b] * 0.5 * (alpha_1^2 + alpha_2^2)[:, None]

Sharding: pure data parallel over the batch dim. B=32 batches are split
4-per-core over 8 NeuronCores; no cross-core communication.

Per-core pipeline (per local batch), memory-roofline bound (~50MB DMA/core):
  - wd streamed in 2MB contiguous chunks -> SBUF [128, 4096]
    (s = 2048*c + 16*p + j; p = partition, j in 0..15)
  - logits on the DVE: one fused scalar_tensor_tensor (product + free-axis
    accumulate) per [128, 256] tile against host-broadcast e_k
    -> psA[128, 64] (A_k per (tile, k) col).
  - softmax stats: row max via DVE reduce + PE transpose + DVE reduce;
    exp on ACT with per-partition accumulate, cross-partition sums via a
    ones-vector matmul, 1/Z on DVE reciprocal.  alpha is assembled as
    c_1*E_1^2 + c_2*E_2^2 with c_k = 0.5/Z_k^2 broadcast across partitions
    by a rank-1 matmul (no Ln -> single ACT table load).
  - out = wM * alpha via per-partition scaled multiply, split ACT/DVE.
  - The per-batch stats chain is a long cross-engine dependency chain, so
    emission is software-pipelined: phase 1 of batch b+1 is emitted before
    stats/finals of batch b, letting the DVE keep streaming dot products
    while batch b's stats hop across engines.
"""

import numpy as np
from contextlib import ExitStack

import concourse.bass as bass
import concourse.bacc as bacc
import concourse.tile as tile
from concourse import mybir
from concourse.bass_utils import run_bass_kernel_spmd

B, S, D = 32, 4096, 256
N_CORES = 8
BPC = B // N_CORES          # batches per core
CHUNK = 2048                # S-rows per DMA chunk (2MB)
NCH = S // CHUNK            # chunks per batch
JP = CHUNK // 128           # 128-row tiles per chunk
NT = S // 128               # 128-row tiles per batch
F32 = mybir.dt.float32
AF = mybir.ActivationFunctionType
ALU = mybir.AluOpType
CORE_IDS = list(range(N_CORES))

_cache: dict = {}


def _build():
    nc = bacc.Bacc("TRN2", target_bir_lowering=False, debug=False,
                   num_devices=N_CORES)
    wd_h = nc.declare_dram_parameter("wd", [BPC, S, D], F32, isOutput=False)
    wM_h = nc.declare_dram_parameter("wM", [BPC, S, D], F32, isOutput=False)
    # ebc[p, ((b*2 + k)*D + d)] = e_k[b, d]  (same for every partition p)
    eb_h = nc.declare_dram_parameter("ebc", [128, BPC * 2 * D], F32,
                                     isOutput=False)
    id_h = nc.declare_dram_parameter("ident", [128, 128], F32, isOutput=False)
    oc_h = nc.declare_dram_parameter("onescol", [128, 1], F32, isOutput=False)
    nr_h = nc.declare_dram_parameter("negonerow", [1, 128], F32, isOutput=False)
    or_h = nc.declare_dram_parameter("onesrow", [1, 128], F32, isOutput=False)
    out_h = nc.declare_dram_parameter("out", [BPC, S, D], F32, isOutput=True)

    def chunk_view(h, b, c):
        # [CHUNK, D] contiguous rows -> [128, JP*D]; s = CHUNK*c + JP*p + j
        return h[b, CHUNK * c:CHUNK * (c + 1), :].rearrange(
            "(p j) d -> p (j d)", p=128)

    with tile.TileContext(nc) as tc, ExitStack() as ctx:
        consts = ctx.enter_context(tc.tile_pool(name="consts", bufs=1))
        wd_pool = ctx.enter_context(tc.tile_pool(name="wdp", bufs=4))
        wm_pool = ctx.enter_context(tc.tile_pool(name="wmp", bufs=3))
        out_pool = ctx.enter_context(tc.tile_pool(name="outp", bufs=3))
        scr_pool = ctx.enter_context(tc.tile_pool(name="scrp", bufs=3))
        sm_pool = ctx.enter_context(tc.tile_pool(name="smalls", bufs=2))
        al_pool = ctx.enter_context(tc.tile_pool(name="alphas", bufs=2))
        pss_pool = ctx.enter_context(tc.tile_pool(name="pss", bufs=2, space="PSUM"))

        # ebc gates the first dot products: load it first, on the load ring.
        ebc = consts.tile([128, BPC * 2 * D], F32)
        nc.sync.dma_start(ebc[:], eb_h[:])
        # small constants only feed the stats chain: use the store ring so
        # they don't delay the first wd chunks.
        ident = consts.tile([128, 128], F32)
        nc.scalar.dma_start(ident[:], id_h[:])
        onescol = consts.tile([128, 1], F32)
        nc.scalar.dma_start(onescol[:], oc_h[:])
        negone = consts.tile([1, 128], F32)
        nc.scalar.dma_start(negone[:], nr_h[:])
        onesrow = consts.tile([1, 128], F32)
        nc.scalar.dma_start(onesrow[:], or_h[:])

        psAs = {}

        def phase1(b):
            # logits: psA[p, 2t+k] = sum_d wd[s(p,t), d] * e_k[d]
            psA = al_pool.tile([128, 2 * NT], F32, tag="psA")
            psAs[b] = psA
            for c in range(NCH):
                wd_ch = wd_pool.tile([128, JP * D], F32, tag="wd")
                nc.sync.dma_start(wd_ch[:], chunk_view(wd_h, b, c))
                for j in range(JP):
                    t = c * JP + j
                    wsl = wd_ch[:, j * D:(j + 1) * D]
                    for k in range(2):
                        scr = scr_pool.tile([128, D], F32, tag="scr")
                        nc.vector.scalar_tensor_tensor(
                            scr[:], wsl, 1.0,
                            ebc[:, (b * 2 + k) * D:(b * 2 + k + 1) * D],
                            op0=ALU.mult, op1=ALU.mult,
                            accum_out=psA[:, 2 * t + k:2 * t + k + 1])

        def phase23(b):
            psA = psAs.pop(b)
            # ---- softmax ----
            mx = sm_pool.tile([128, 1], F32, tag="mx")
            nc.vector.tensor_reduce(mx[:], psA[:], axis=mybir.AxisListType.X,
                                    op=ALU.max)
            tmax = pss_pool.tile([1, 128], F32, tag="pssm")
            nc.tensor.transpose(tmax[:], mx[:], ident[:])
            m2 = sm_pool.tile([1, 1], F32, tag="m2")
            nc.vector.tensor_reduce(m2[:], tmax[:], axis=mybir.AxisListType.X,
                                    op=ALU.max)
            # broadcast -maxA to all partitions: [128,1] psum -> sbuf
            mneg_ps = pss_pool.tile([128, 1], F32, tag="pssm")
            nc.tensor.matmul(mneg_ps[:], negone[:], m2[:], start=True, stop=True)
            mneg = sm_pool.tile([128, 1], F32, tag="mneg")
            nc.scalar.copy(mneg[:], mneg_ps[:])

            # E_k = exp(A_k - maxA) with per-partition accumulate
            psA_v = psA[:].rearrange("p (t k) -> p k t", k=2)
            E = al_pool.tile([128, 2 * NT], F32, tag="E")
            E_v = E[:].rearrange("p (t k) -> p k t", k=2)
            s12 = sm_pool.tile([128, 2], F32, tag="s12")
            for k in range(2):
                nc.scalar.activation(E_v[:, k, :], psA_v[:, k, :], AF.Exp,
                                     bias=mneg[:], scale=1.0,
                                     accum_out=s12[:, k:k + 1])
            # cross-partition sum -> Z'[1,2]; c_k = 0.5 / Z'_k^2
            zsum = pss_pool.tile([1, 2], F32, tag="pssm")
            nc.tensor.matmul(zsum[:], onescol[:], s12[:], start=True, stop=True)
            zinv = sm_pool.tile([1, 2], F32, tag="zinv")
            nc.vector.reciprocal(zinv[:], zsum[:])
            zz = sm_pool.tile([1, 2], F32, tag="zz")
            nc.vector.tensor_scalar(zz[:], zinv[:], 0.5, None, op0=ALU.mult)
            nc.vector.tensor_mul(zz[:], zz[:], zinv[:])
            # broadcast c_k to all partitions: [128, 2] psum -> sbuf
            c_ps = pss_pool.tile([128, 2], F32, tag="pssm")
            nc.tensor.matmul(c_ps[:], onesrow[:], zz[:], start=True, stop=True)
            c12 = sm_pool.tile([128, 2], F32, tag="c12")
            nc.scalar.copy(c12[:], c_ps[:])

            # alpha = c_1*E_1^2 + c_2*E_2^2
            esq = al_pool.tile([128, 2 * NT], F32, tag="esq")
            nc.vector.tensor_mul(esq[:], E[:], E[:])
            esq_v = esq[:].rearrange("p (t k) -> p k t", k=2)
            atmp = al_pool.tile([128, NT], F32, tag="atmp")
            nc.vector.tensor_scalar_mul(atmp[:], esq_v[:, 1, :], c12[:, 1:2])
            alpha = al_pool.tile([128, NT], F32, tag="alpha")
            nc.vector.scalar_tensor_tensor(alpha[:], esq_v[:, 0, :],
                                           c12[:, 0:1], atmp[:],
                                           op0=ALU.mult, op1=ALU.add)

            # ---- out = wM * alpha ----
            # ACT is cheaper to spare than DVE late in the kernel: the last
            # batch leans on the DVE (194ns/tile vs ~660ns on ACT).
            for c in range(NCH):
                wm_ch = wm_pool.tile([128, JP * D], F32, tag="wm")
                nc.sync.dma_start(wm_ch[:], chunk_view(wM_h, b, c))
                out_ch = out_pool.tile([128, JP * D], F32, tag="out")
                for j in range(JP):
                    t = c * JP + j
                    sl = slice(j * D, (j + 1) * D)
                    if j % 2 == 0 or b == BPC - 1:
                        nc.vector.tensor_scalar_mul(out_ch[:, sl],
                                                    wm_ch[:, sl],
                                                    alpha[:, t:t + 1])
                    else:
                        nc.scalar.mul(out_ch[:, sl], wm_ch[:, sl],
                                      alpha[:, t:t + 1])
                nc.scalar.dma_start(chunk_view(out_h, b, c), out_ch[:])

        # software pipeline: keep the DVE busy with batch b+1's dot products
        # while batch b's stats chain hops across engines.
        phase1(0)
        for b in range(BPC):
            if b + 1 < BPC:
                phase1(b + 1)
            phase23(b)

    nc.finalize()
    return nc


def _get_nc():
    if "nc" not in _cache:
        _cache["nc"] = _build()
    return _cache["nc"]


def _in_maps(wM, wd, e1, e2):
    ident = np.eye(128, dtype=np.float32)
    onescol = np.ones((128, 1), dtype=np.float32)
    negone = np.full((1, 128), -1.0, dtype=np.float32)
    onesrow = np.ones((1, 128), dtype=np.float32)
    maps = []
    for i in range(N_CORES):
        sl = slice(i * BPC, (i + 1) * BPC)
        # [BPC, 2, D] -> row vector repeated over 128 partitions
        ebc = np.stack([e1[sl], e2[sl]], axis=1).reshape(1, BPC * 2 * D)
        ebc = np.ascontiguousarray(np.broadcast_to(ebc, (128, BPC * 2 * D)))
        maps.append({
            "wd": np.ascontiguousarray(wd[sl]),
            "wM": np.ascontiguousarray(wM[sl]),
            "ebc": ebc,
            "ident": ident,
            "onescol": onescol,
            "negonerow": negone,
            "onesrow": onesrow,
        })
    return maps


def _run(wM, wd, e1, e2, **kw):
    wM = np.asarray(wM, dtype=np.float32)
    wd = np.asarray(wd, dtype=np.float32)
    e1 = np.asarray(e1, dtype=np.float32)
    e2 = np.asarray(e2, dtype=np.float32)
    nc = _get_nc()
    res = run_bass_kernel_spmd(nc, _in_maps(wM, wd, e1, e2), CORE_IDS, **kw)
    out = np.concatenate([r["out"] for r in res.results], axis=0)
    return out, res


def kernel(wM, wd, e1, e2):
    out, _ = _run(wM, wd, e1, e2)
    return out


# revision 16
# speedup vs baseline: 1.4830x; 1.0325x over previous
"""Trainium2 Bass kernel for entity-attention input scaling.

Computes, per batch row b:
    A_k = wd[b] @ e_k[b]          (k = 1, 2)   [S]
    alpha_k = softmax(A_k)
    out[b]  = wM[b] * 0.5 * (alpha_1^2 + alpha_2^2)[:, None]

Sharding: pure data parallel over the batch dim. B=32 batches are split
4-per-core over 8 NeuronCores; no cross-core communication.

Per-core pipeline (per local batch), memory-roofline bound (~50MB DMA/core):
  - wd streamed in 2MB contiguous chunks -> SBUF [128, 4096]
    (s = 2048*c + 16*p + j; p = partition, j in 0..15)
  - logits on the DVE: one fused scalar_tensor_tensor (product + free-axis
    accumulate) per [128, 256] tile against host-broadcast e_k
    -> psA[128, 64] (A_k per (tile, k) col).
  - softmax stats: row max via DVE reduce + PE transpose + DVE reduce;
    exp on ACT with per-partition accumulate, cross-partition sums via a
    ones-vector matmul, 1/Z on DVE reciprocal.  alpha is assembled as
    c_1*E_1^2 + c_2*E_2^2 with c_k = 0.5/Z_k^2 broadcast across partitions
    by a rank-1 matmul (no Ln -> single ACT table load).
  - out = wM * alpha via per-partition scaled multiply, split ACT/DVE.
  - The per-batch stats chain is a long cross-engine dependency chain, so
    emission is software-pipelined: phase 1 of batch b+1 is emitted before
    stats/finals of batch b, letting the DVE keep streaming dot products
    while batch b's stats hop across engines.
"""

import numpy as np
from contextlib import ExitStack

import concourse.bass as bass
import concourse.bacc as bacc
import concourse.tile as tile
from concourse import mybir
from concourse.bass_utils import run_bass_kernel_spmd

B, S, D = 32, 4096, 256
N_CORES = 8
BPC = B // N_CORES          # batches per core
CHUNK = 2048                # S-rows per DMA chunk (2MB)
NCH = S // CHUNK            # chunks per batch
JP = CHUNK // 128           # 128-row tiles per chunk
NT = S // 128               # 128-row tiles per batch
F32 = mybir.dt.float32
AF = mybir.ActivationFunctionType
ALU = mybir.AluOpType
CORE_IDS = list(range(N_CORES))

_cache: dict = {}


def _build():
    nc = bacc.Bacc("TRN2", target_bir_lowering=False, debug=False,
                   num_devices=N_CORES)
    wd_h = nc.declare_dram_parameter("wd", [BPC, S, D], F32, isOutput=False)
    wM_h = nc.declare_dram_parameter("wM", [BPC, S, D], F32, isOutput=False)
    # ebc[p, ((b*2 + k)*D + d)] = e_k[b, d]  (same for every partition p)
    eb_h = nc.declare_dram_parameter("ebc", [128, BPC * 2 * D], F32,
                                     isOutput=False)
    id_h = nc.declare_dram_parameter("ident", [128, 128], F32, isOutput=False)
    oc_h = nc.declare_dram_parameter("onescol", [128, 1], F32, isOutput=False)
    nr_h = nc.declare_dram_parameter("negonerow", [1, 128], F32, isOutput=False)
    or_h = nc.declare_dram_parameter("onesrow", [1, 128], F32, isOutput=False)
    out_h = nc.declare_dram_parameter("out", [BPC, S, D], F32, isOutput=True)

    def chunk_view(h, b, c):
        # [CHUNK, D] contiguous rows -> [128, JP*D]; s = CHUNK*c + JP*p + j
        return h[b, CHUNK * c:CHUNK * (c + 1), :].rearrange(
            "(p j) d -> p (j d)", p=128)

    with tile.TileContext(nc) as tc, ExitStack() as ctx:
        consts = ctx.enter_context(tc.tile_pool(name="consts", bufs=1))
        wd_pool = ctx.enter_context(tc.tile_pool(name="wdp", bufs=4))
        wm_pool = ctx.enter_context(tc.tile_pool(name="wmp", bufs=3))
        out_pool = ctx.enter_context(tc.tile_pool(name="outp", bufs=3))
        scr_pool = ctx.enter_context(tc.tile_pool(name="scrp", bufs=3))
        sm_pool = ctx.enter_context(tc.tile_pool(name="smalls", bufs=2))
        al_pool = ctx.enter_context(tc.tile_pool(name="alphas", bufs=2))
        pss_pool = ctx.enter_context(tc.tile_pool(name="pss", bufs=2, space="PSUM"))

        # ebc gates the first dot products: load it on the store ring (idle
        # at kernel start) so it runs in parallel with the first wd chunk.
        ebc = consts.tile([128, BPC * 2 * D], F32)
        nc.scalar.dma_start(ebc[:], eb_h[:])
        # small constants only feed the stats chain: use the store ring so
        # they don't delay the first wd chunks.
        ident = consts.tile([128, 128], F32)
        nc.scalar.dma_start(ident[:], id_h[:])
        onescol = consts.tile([128, 1], F32)
        nc.scalar.dma_start(onescol[:], oc_h[:])
        negone = consts.tile([1, 128], F32)
        nc.scalar.dma_start(negone[:], nr_h[:])
        onesrow = consts.tile([1, 128], F32)
        nc.scalar.dma_start(onesrow[:], or_h[:])

        psAs = {}

        def phase1(b):
            # logits: psA[p, 2t+k] = sum_d wd[s(p,t), d] * e_k[d]
            psA = al_pool.tile([128, 2 * NT], F32, tag="psA")
            psAs[b] = psA
            for c in range(NCH):
                # First chunk of the kernel arrives in 1MB quarters so the
                # DVE starts ~6us earlier; steady state uses one 2MB DMA.
                nparts = 4 if (b == 0 and c == 0) else 1
                jpp = JP // nparts
                wd_ch = wd_pool.tile([128, JP * D], F32, tag="wd")
                full = chunk_view(wd_h, b, c)
                for p_ in range(nparts):
                    fsl = slice(p_ * jpp * D, (p_ + 1) * jpp * D)
                    nc.sync.dma_start(wd_ch[:, fsl], full[:, fsl])
                for j in range(JP):
                    t = c * JP + j
                    wsl = wd_ch[:, j * D:(j + 1) * D]
                    for k in range(2):
                        scr = scr_pool.tile([128, D], F32, tag="scr")
                        nc.vector.scalar_tensor_tensor(
                            scr[:], wsl, 1.0,
                            ebc[:, (b * 2 + k) * D:(b * 2 + k + 1) * D],
                            op0=ALU.mult, op1=ALU.mult,
                            accum_out=psA[:, 2 * t + k:2 * t + k + 1])

        def phase23(b):
            psA = psAs.pop(b)
            # ---- softmax ----
            mx = sm_pool.tile([128, 1], F32, tag="mx")
            nc.vector.tensor_reduce(mx[:], psA[:], axis=mybir.AxisListType.X,
                                    op=ALU.max)
            tmax = pss_pool.tile([1, 128], F32, tag="pssm")
            nc.tensor.transpose(tmax[:], mx[:], ident[:])
            m2 = sm_pool.tile([1, 1], F32, tag="m2")
            nc.vector.tensor_reduce(m2[:], tmax[:], axis=mybir.AxisListType.X,
                                    op=ALU.max)
            # broadcast -maxA to all partitions: [128,1] psum -> sbuf
            mneg_ps = pss_pool.tile([128, 1], F32, tag="pssm")
            nc.tensor.matmul(mneg_ps[:], negone[:], m2[:], start=True, stop=True)
            mneg = sm_pool.tile([128, 1], F32, tag="mneg")
            nc.scalar.copy(mneg[:], mneg_ps[:])

            # E_k = exp(A_k - maxA) with per-partition accumulate
            psA_v = psA[:].rearrange("p (t k) -> p k t", k=2)
            E = al_pool.tile([128, 2 * NT], F32, tag="E")
            E_v = E[:].rearrange("p (t k) -> p k t", k=2)
            s12 = sm_pool.tile([128, 2], F32, tag="s12")
            for k in range(2):
                nc.scalar.activation(E_v[:, k, :], psA_v[:, k, :], AF.Exp,
                                     bias=mneg[:], scale=1.0,
                                     accum_out=s12[:, k:k + 1])
            # cross-partition sum -> Z'[1,2]; c_k = 0.5 / Z'_k^2
            zsum = pss_pool.tile([1, 2], F32, tag="pssm")
            nc.tensor.matmul(zsum[:], onescol[:], s12[:], start=True, stop=True)
            zinv = sm_pool.tile([1, 2], F32, tag="zinv")
            nc.vector.reciprocal(zinv[:], zsum[:])
            zz = sm_pool.tile([1, 2], F32, tag="zz")
            nc.vector.tensor_scalar(zz[:], zinv[:], 0.5, None, op0=ALU.mult)
            nc.vector.tensor_mul(zz[:], zz[:], zinv[:])
            # broadcast c_k to all partitions: [128, 2] psum -> sbuf
            c_ps = pss_pool.tile([128, 2], F32, tag="pssm")
            nc.tensor.matmul(c_ps[:], onesrow[:], zz[:], start=True, stop=True)
            c12 = sm_pool.tile([128, 2], F32, tag="c12")
            nc.scalar.copy(c12[:], c_ps[:])

            # alpha = c_1*E_1^2 + c_2*E_2^2
            esq = al_pool.tile([128, 2 * NT], F32, tag="esq")
            nc.vector.tensor_mul(esq[:], E[:], E[:])
            esq_v = esq[:].rearrange("p (t k) -> p k t", k=2)
            atmp = al_pool.tile([128, NT], F32, tag="atmp")
            nc.vector.tensor_scalar_mul(atmp[:], esq_v[:, 1, :], c12[:, 1:2])
            alpha = al_pool.tile([128, NT], F32, tag="alpha")
            nc.vector.scalar_tensor_tensor(alpha[:], esq_v[:, 0, :],
                                           c12[:, 0:1], atmp[:],
                                           op0=ALU.mult, op1=ALU.add)

            # ---- out = wM * alpha ----
            # ACT is cheaper to spare than DVE late in the kernel: the last
            # batch leans on the DVE (194ns/tile vs ~660ns on ACT).
            # The last batch's wM lands last on the load ring and its stores
            # are the kernel tail: stream it in 1MB quarters so finals and
            # stores pipeline tightly behind the loads.
            nparts = 4 if b == BPC - 1 else 1
            jpp = JP // nparts
            for c in range(NCH):
                wm_ch = wm_pool.tile([128, JP * D], F32, tag="wm")
                out_ch = out_pool.tile([128, JP * D], F32, tag="out")
                wm_full = chunk_view(wM_h, b, c)
                out_full = chunk_view(out_h, b, c)
                for p_ in range(nparts):
                    fsl = slice(p_ * jpp * D, (p_ + 1) * jpp * D)
                    nc.sync.dma_start(wm_ch[:, fsl], wm_full[:, fsl])
                    for j in range(p_ * jpp, (p_ + 1) * jpp):
                        t = c * JP + j
                        sl = slice(j * D, (j + 1) * D)
                        if j % 2 == 0 or b == BPC - 1:
                            nc.vector.tensor_scalar_mul(out_ch[:, sl],
                                                        wm_ch[:, sl],
                                                        alpha[:, t:t + 1])
                        else:
                            nc.scalar.mul(out_ch[:, sl], wm_ch[:, sl],
                                          alpha[:, t:t + 1])
                    nc.scalar.dma_start(out_full[:, fsl], out_ch[:, fsl])

        # software pipeline: keep the DVE busy with batch b+1's dot products
        # while batch b's stats chain hops across engines.
        phase1(0)
        for b in range(BPC):
            if b + 1 < BPC:
                phase1(b + 1)
            phase23(b)

    nc.finalize()
    return nc


def _get_nc():
    if "nc" not in _cache:
        _cache["nc"] = _build()
    return _cache["nc"]


def _in_maps(wM, wd, e1, e2):
    ident = np.eye(128, dtype=np.float32)
    onescol = np.ones((128, 1), dtype=np.float32)
    negone = np.full((1, 128), -1.0, dtype=np.float32)
    onesrow = np.ones((1, 128), dtype=np.float32)
    maps = []
    for i in range(N_CORES):
        sl = slice(i * BPC, (i + 1) * BPC)
        # [BPC, 2, D] -> row vector repeated over 128 partitions
        ebc = np.stack([e1[sl], e2[sl]], axis=1).reshape(1, BPC * 2 * D)
        ebc = np.ascontiguousarray(np.broadcast_to(ebc, (128, BPC * 2 * D)))
        maps.append({
            "wd": np.ascontiguousarray(wd[sl]),
            "wM": np.ascontiguousarray(wM[sl]),
            "ebc": ebc,
            "ident": ident,
            "onescol": onescol,
            "negonerow": negone,
            "onesrow": onesrow,
        })
    return maps


def _run(wM, wd, e1, e2, **kw):
    wM = np.asarray(wM, dtype=np.float32)
    wd = np.asarray(wd, dtype=np.float32)
    e1 = np.asarray(e1, dtype=np.float32)
    e2 = np.asarray(e2, dtype=np.float32)
    nc = _get_nc()
    res = run_bass_kernel_spmd(nc, _in_maps(wM, wd, e1, e2), CORE_IDS, **kw)
    out = np.concatenate([r["out"] for r in res.results], axis=0)
    return out, res


def kernel(wM, wd, e1, e2):
    out, _ = _run(wM, wd, e1, e2)
    return out


# revision 17
# speedup vs baseline: 1.5469x; 1.0431x over previous
"""Trainium2 Bass kernel for entity-attention input scaling.

Computes, per batch row b:
    A_k = wd[b] @ e_k[b]          (k = 1, 2)   [S]
    alpha_k = softmax(A_k)
    out[b]  = wM[b] * 0.5 * (alpha_1^2 + alpha_2^2)[:, None]

Sharding: pure data parallel over the batch dim. B=32 batches are split
4-per-core over 8 NeuronCores; no cross-core communication.

Per-core pipeline (per local batch), memory-roofline bound (~50MB DMA/core):
  - wd streamed in 2MB contiguous chunks -> SBUF [128, 4096]
    (s = 2048*c + 16*p + j; p = partition, j in 0..15)
  - logits on the DVE: one fused scalar_tensor_tensor (product + free-axis
    accumulate) per [128, 256] tile against host-broadcast e_k
    -> psA[128, 64] (A_k per (tile, k) col).
  - softmax stats: row max via DVE reduce + PE transpose + DVE reduce;
    exp on ACT with per-partition accumulate, cross-partition sums via a
    ones-vector matmul, 1/Z on DVE reciprocal.  alpha is assembled as
    c_1*E_1^2 + c_2*E_2^2 with c_k = 0.5/Z_k^2 broadcast across partitions
    by a rank-1 matmul (no Ln -> single ACT table load).
  - out = wM * alpha via per-partition scaled multiply, split ACT/DVE.
  - The per-batch stats chain is a long cross-engine dependency chain, so
    emission is software-pipelined: phase 1 of batch b+1 is emitted before
    stats/finals of batch b, letting the DVE keep streaming dot products
    while batch b's stats hop across engines.
"""

import numpy as np
from contextlib import ExitStack

import concourse.bass as bass
import concourse.bacc as bacc
import concourse.tile as tile
from concourse import mybir
from concourse.bass_utils import run_bass_kernel_spmd

B, S, D = 32, 4096, 256
N_CORES = 8
BPC = B // N_CORES          # batches per core
CHUNK = 2048                # S-rows per DMA chunk (2MB)
NCH = S // CHUNK            # chunks per batch
JP = CHUNK // 128           # 128-row tiles per chunk
NT = S // 128               # 128-row tiles per batch
F32 = mybir.dt.float32
AF = mybir.ActivationFunctionType
ALU = mybir.AluOpType
CORE_IDS = list(range(N_CORES))

_cache: dict = {}


def _build():
    nc = bacc.Bacc("TRN2", target_bir_lowering=False, debug=False,
                   num_devices=N_CORES)
    wd_h = nc.declare_dram_parameter("wd", [BPC, S, D], F32, isOutput=False)
    wM_h = nc.declare_dram_parameter("wM", [BPC, S, D], F32, isOutput=False)
    # ebc[p, ((b*2 + k)*D + d)] = e_k[b, d]  (same for every partition p)
    eb_h = nc.declare_dram_parameter("ebc", [128, BPC * 2 * D], F32,
                                     isOutput=False)
    id_h = nc.declare_dram_parameter("ident", [128, 128], F32, isOutput=False)
    oc_h = nc.declare_dram_parameter("onescol", [128, 1], F32, isOutput=False)
    nr_h = nc.declare_dram_parameter("negonerow", [1, 128], F32, isOutput=False)
    or_h = nc.declare_dram_parameter("onesrow", [1, 128], F32, isOutput=False)
    out_h = nc.declare_dram_parameter("out", [BPC, S, D], F32, isOutput=True)

    def chunk_view(h, b, c):
        # [CHUNK, D] contiguous rows -> [128, JP*D]; s = CHUNK*c + JP*p + j
        return h[b, CHUNK * c:CHUNK * (c + 1), :].rearrange(
            "(p j) d -> p (j d)", p=128)

    with tile.TileContext(nc) as tc, ExitStack() as ctx:
        consts = ctx.enter_context(tc.tile_pool(name="consts", bufs=1))
        wd_pool = ctx.enter_context(tc.tile_pool(name="wdp", bufs=3))
        wm_pool = ctx.enter_context(tc.tile_pool(name="wmp", bufs=4))
        out_pool = ctx.enter_context(tc.tile_pool(name="outp", bufs=3))
        scr_pool = ctx.enter_context(tc.tile_pool(name="scrp", bufs=2))
        sm_pool = ctx.enter_context(tc.tile_pool(name="smalls", bufs=2))
        al_pool = ctx.enter_context(tc.tile_pool(name="alphas", bufs=2))
        pss_pool = ctx.enter_context(tc.tile_pool(name="pss", bufs=2, space="PSUM"))

        # ebc gates the first dot products: load it on the store ring (idle
        # at kernel start) so it runs in parallel with the first wd chunk.
        ebc = consts.tile([128, BPC * 2 * D], F32)
        nc.scalar.dma_start(ebc[:], eb_h[:])
        # small constants only feed the stats chain: use the store ring so
        # they don't delay the first wd chunks.
        ident = consts.tile([128, 128], F32)
        nc.scalar.dma_start(ident[:], id_h[:])
        onescol = consts.tile([128, 1], F32)
        nc.scalar.dma_start(onescol[:], oc_h[:])
        negone = consts.tile([1, 128], F32)
        nc.scalar.dma_start(negone[:], nr_h[:])
        onesrow = consts.tile([1, 128], F32)
        nc.scalar.dma_start(onesrow[:], or_h[:])

        psAs = {}

        def phase1(b):
            # logits: psA[p, 2t+k] = sum_d wd[s(p,t), d] * e_k[d]
            psA = al_pool.tile([128, 2 * NT], F32, tag="psA")
            psAs[b] = psA
            for c in range(NCH):
                # First chunk of the kernel arrives in 1MB quarters so the
                # DVE starts ~6us earlier; steady state uses one 2MB DMA.
                nparts = 4 if (b == 0 and c == 0) else 1
                jpp = JP // nparts
                wd_ch = wd_pool.tile([128, JP * D], F32, tag="wd")
                full = chunk_view(wd_h, b, c)
                for p_ in range(nparts):
                    fsl = slice(p_ * jpp * D, (p_ + 1) * jpp * D)
                    nc.sync.dma_start(wd_ch[:, fsl], full[:, fsl])
                for j in range(JP):
                    t = c * JP + j
                    wsl = wd_ch[:, j * D:(j + 1) * D]
                    for k in range(2):
                        scr = scr_pool.tile([128, D], F32, tag="scr")
                        nc.vector.scalar_tensor_tensor(
                            scr[:], wsl, 1.0,
                            ebc[:, (b * 2 + k) * D:(b * 2 + k + 1) * D],
                            op0=ALU.mult, op1=ALU.mult,
                            accum_out=psA[:, 2 * t + k:2 * t + k + 1])

        def phase23(b):
            psA = psAs.pop(b)
            # ---- softmax ----
            mx = sm_pool.tile([128, 1], F32, tag="mx")
            nc.vector.tensor_reduce(mx[:], psA[:], axis=mybir.AxisListType.X,
                                    op=ALU.max)
            tmax = pss_pool.tile([1, 128], F32, tag="pssm")
            nc.tensor.transpose(tmax[:], mx[:], ident[:])
            m2 = sm_pool.tile([1, 1], F32, tag="m2")
            nc.vector.tensor_reduce(m2[:], tmax[:], axis=mybir.AxisListType.X,
                                    op=ALU.max)
            # broadcast -maxA to all partitions: [128,1] psum -> sbuf
            mneg_ps = pss_pool.tile([128, 1], F32, tag="pssm")
            nc.tensor.matmul(mneg_ps[:], negone[:], m2[:], start=True, stop=True)
            mneg = sm_pool.tile([128, 1], F32, tag="mneg")
            nc.scalar.copy(mneg[:], mneg_ps[:])

            # E_k = exp(A_k - maxA) with per-partition accumulate
            psA_v = psA[:].rearrange("p (t k) -> p k t", k=2)
            E = al_pool.tile([128, 2 * NT], F32, tag="E")
            E_v = E[:].rearrange("p (t k) -> p k t", k=2)
            s12 = sm_pool.tile([128, 2], F32, tag="s12")
            for k in range(2):
                nc.scalar.activation(E_v[:, k, :], psA_v[:, k, :], AF.Exp,
                                     bias=mneg[:], scale=1.0,
                                     accum_out=s12[:, k:k + 1])
            # cross-partition sum -> Z'[1,2]; c_k = 0.5 / Z'_k^2
            zsum = pss_pool.tile([1, 2], F32, tag="pssm")
            nc.tensor.matmul(zsum[:], onescol[:], s12[:], start=True, stop=True)
            zinv = sm_pool.tile([1, 2], F32, tag="zinv")
            nc.vector.reciprocal(zinv[:], zsum[:])
            zz = sm_pool.tile([1, 2], F32, tag="zz")
            nc.vector.tensor_scalar(zz[:], zinv[:], 0.5, None, op0=ALU.mult)
            nc.vector.tensor_mul(zz[:], zz[:], zinv[:])
            # broadcast c_k to all partitions: [128, 2] psum -> sbuf
            c_ps = pss_pool.tile([128, 2], F32, tag="pssm")
            nc.tensor.matmul(c_ps[:], onesrow[:], zz[:], start=True, stop=True)
            c12 = sm_pool.tile([128, 2], F32, tag="c12")
            nc.scalar.copy(c12[:], c_ps[:])

            # alpha = c_1*E_1^2 + c_2*E_2^2
            esq = al_pool.tile([128, 2 * NT], F32, tag="esq")
            nc.vector.tensor_mul(esq[:], E[:], E[:])
            esq_v = esq[:].rearrange("p (t k) -> p k t", k=2)
            atmp = al_pool.tile([128, NT], F32, tag="atmp")
            nc.vector.tensor_scalar_mul(atmp[:], esq_v[:, 1, :], c12[:, 1:2])
            alpha = al_pool.tile([128, NT], F32, tag="alpha")
            nc.vector.scalar_tensor_tensor(alpha[:], esq_v[:, 0, :],
                                           c12[:, 0:1], atmp[:],
                                           op0=ALU.mult, op1=ALU.add)

            # ---- out = wM * alpha ----
            # ACT is cheaper to spare than DVE late in the kernel: the last
            # batch leans on the DVE (194ns/tile vs ~660ns on ACT).
            # The last batch's wM lands last on the load ring and its stores
            # are the kernel tail: stream it in 1MB quarters so finals and
            # stores pipeline tightly behind the loads.
            nparts = 4 if b == BPC - 1 else 1
            jpp = JP // nparts
            for c in range(NCH):
                wm_ch = wm_pool.tile([128, JP * D], F32, tag="wm")
                out_ch = out_pool.tile([128, JP * D], F32, tag="out")
                wm_full = chunk_view(wM_h, b, c)
                out_full = chunk_view(out_h, b, c)
                for p_ in range(nparts):
                    fsl = slice(p_ * jpp * D, (p_ + 1) * jpp * D)
                    nc.sync.dma_start(wm_ch[:, fsl], wm_full[:, fsl])
                    for j in range(p_ * jpp, (p_ + 1) * jpp):
                        t = c * JP + j
                        sl = slice(j * D, (j + 1) * D)
                        if j % 2 == 0 or b == BPC - 1:
                            nc.vector.tensor_scalar_mul(out_ch[:, sl],
                                                        wm_ch[:, sl],
                                                        alpha[:, t:t + 1])
                        else:
                            nc.scalar.mul(out_ch[:, sl], wm_ch[:, sl],
                                          alpha[:, t:t + 1])
                    nc.scalar.dma_start(out_full[:, fsl], out_ch[:, fsl])

        # software pipeline: keep the DVE busy with batch b+1's dot products
        # while batch b's stats chain hops across engines.
        phase1(0)
        for b in range(BPC):
            if b + 1 < BPC:
                phase1(b + 1)
            phase23(b)

    nc.finalize()
    return nc


def _get_nc():
    if "nc" not in _cache:
        _cache["nc"] = _build()
    return _cache["nc"]


def _in_maps(wM, wd, e1, e2):
    ident = np.eye(128, dtype=np.float32)
    onescol = np.ones((128, 1), dtype=np.float32)
    negone = np.full((1, 128), -1.0, dtype=np.float32)
    onesrow = np.ones((1, 128), dtype=np.float32)
    maps = []
    for i in range(N_CORES):
        sl = slice(i * BPC, (i + 1) * BPC)
        # [BPC, 2, D] -> row vector repeated over 128 partitions
        ebc = np.stack([e1[sl], e2[sl]], axis=1).reshape(1, BPC * 2 * D)
        ebc = np.ascontiguousarray(np.broadcast_to(ebc, (128, BPC * 2 * D)))
        maps.append({
            "wd": np.ascontiguousarray(wd[sl]),
            "wM": np.ascontiguousarray(wM[sl]),
            "ebc": ebc,
            "ident": ident,
            "onescol": onescol,
            "negonerow": negone,
            "onesrow": onesrow,
        })
    return maps


def _run(wM, wd, e1, e2, **kw):
    wM = np.asarray(wM, dtype=np.float32)
    wd = np.asarray(wd, dtype=np.float32)
    e1 = np.asarray(e1, dtype=np.float32)
    e2 = np.asarray(e2, dtype=np.float32)
    nc = _get_nc()
    res = run_bass_kernel_spmd(nc, _in_maps(wM, wd, e1, e2), CORE_IDS, **kw)
    out = np.concatenate([r["out"] for r in res.results], axis=0)
    return out, res


def kernel(wM, wd, e1, e2):
    out, _ = _run(wM, wd, e1, e2)
    return out


# revision 22
# speedup vs baseline: 1.5955x; 1.0314x over previous
"""Trainium2 Bass kernel for entity-attention input scaling.

Computes, per batch row b:
    A_k = wd[b] @ e_k[b]          (k = 1, 2)   [S]
    alpha_k = softmax(A_k)
    out[b]  = wM[b] * 0.5 * (alpha_1^2 + alpha_2^2)[:, None]

Sharding: pure data parallel over the batch dim. B=32 batches are split
4-per-core over 8 NeuronCores; no cross-core communication.

Per-core pipeline (per local batch), memory-roofline bound (~50MB DMA/core):
  - wd streamed in 2MB contiguous chunks -> SBUF [128, 4096]
    (s = 2048*c + 16*p + j; p = partition, j in 0..15)
  - logits on the DVE: one fused scalar_tensor_tensor (product + free-axis
    accumulate) per [128, 256] tile against host-broadcast e_k
    -> psA[128, 64] (A_k per (tile, k) col).
  - softmax stats: row max via DVE reduce + PE transpose + DVE reduce;
    exp on ACT with per-partition accumulate, cross-partition sums via a
    ones-vector matmul, 1/Z on DVE reciprocal.  alpha is assembled as
    c_1*E_1^2 + c_2*E_2^2 with c_k = 0.5/Z_k^2 broadcast across partitions
    by a rank-1 matmul (no Ln -> single ACT table load).
  - out = wM * alpha via per-partition scaled multiply, split ACT/DVE.
  - The per-batch stats chain is a long cross-engine dependency chain, so
    emission is software-pipelined: phase 1 of batch b+1 is emitted before
    stats/finals of batch b, letting the DVE keep streaming dot products
    while batch b's stats hop across engines.
"""

import numpy as np
from contextlib import ExitStack

import concourse.bacc as bacc
import concourse.tile as tile
from concourse import mybir
from concourse.bass_utils import run_bass_kernel_spmd

B, S, D = 32, 4096, 256
N_CORES = 8
BPC = B // N_CORES          # batches per core
CHUNK = 2048                # S-rows per DMA chunk (2MB)
NCH = S // CHUNK            # chunks per batch
JP = CHUNK // 128           # 128-row tiles per chunk
NT = S // 128               # 128-row tiles per batch
F32 = mybir.dt.float32
AF = mybir.ActivationFunctionType
ALU = mybir.AluOpType
CORE_IDS = list(range(N_CORES))

_cache: dict = {}


def _build():
    nc = bacc.Bacc("TRN2", target_bir_lowering=False, debug=False,
                   num_devices=N_CORES)
    wd_h = nc.declare_dram_parameter("wd", [BPC, S, D], F32, isOutput=False)
    wM_h = nc.declare_dram_parameter("wM", [BPC, S, D], F32, isOutput=False)
    # erow[0, ((b*2 + k)*D + d)] = e_k[b, d]; broadcast on-chip (8KB DMA
    # instead of a 1MB pre-broadcast copy)
    er_h = nc.declare_dram_parameter("erow", [1, BPC * 2 * D], F32,
                                     isOutput=False)
    id_h = nc.declare_dram_parameter("ident", [128, 128], F32, isOutput=False)
    out_h = nc.declare_dram_parameter("out", [BPC, S, D], F32, isOutput=True)

    def chunk_view(h, b, c):
        # [CHUNK, D] contiguous rows -> [128, JP*D]; s = CHUNK*c + JP*p + j
        return h[b, CHUNK * c:CHUNK * (c + 1), :].rearrange(
            "(p j) d -> p (j d)", p=128)

    with tile.TileContext(nc) as tc, ExitStack() as ctx:
        consts = ctx.enter_context(tc.tile_pool(name="consts", bufs=1))
        wd_pool = ctx.enter_context(tc.tile_pool(name="wdp", bufs=3))
        wm_pool = ctx.enter_context(tc.tile_pool(name="wmp", bufs=4))
        out_pool = ctx.enter_context(tc.tile_pool(name="outp", bufs=3))
        scr_pool = ctx.enter_context(tc.tile_pool(name="scrp", bufs=2))
        sm_pool = ctx.enter_context(tc.tile_pool(name="smalls", bufs=2))
        al_pool = ctx.enter_context(tc.tile_pool(name="alphas", bufs=2))
        pss_pool = ctx.enter_context(tc.tile_pool(name="pss", bufs=2, space="PSUM"))
        psb_pool = ctx.enter_context(tc.tile_pool(name="psb", bufs=2, space="PSUM"))

        # constants: memset where possible, tiny DMAs on the store ring
        # (idle at kernel start) so nothing delays the first wd chunks.
        onescol = consts.tile([128, 1], F32)
        nc.gpsimd.memset(onescol[:], 1.0)
        onesrow = consts.tile([1, 128], F32)
        nc.gpsimd.memset(onesrow[:], 1.0)
        negone = consts.tile([1, 128], F32)
        nc.gpsimd.memset(negone[:], -1.0)
        ident = consts.tile([128, 128], F32)
        nc.scalar.dma_start(ident[:], id_h[:])
        # e rows: 8KB DMA, then rank-1 matmul broadcast to all partitions
        erow = consts.tile([1, BPC * 2 * D], F32)
        nc.scalar.dma_start(erow[:], er_h[:])
        ebc = consts.tile([128, BPC * 2 * D], F32)
        for q in range(BPC * 2 * D // 512):
            qsl = slice(q * 512, (q + 1) * 512)
            eb_ps = psb_pool.tile([128, 512], F32, tag="ebps")
            nc.tensor.matmul(eb_ps[:], onesrow[:], erow[:, qsl],
                             start=True, stop=True)
            nc.scalar.copy(ebc[:, qsl], eb_ps[:])

        psAs = {}

        def phase1(b):
            # logits: psA[p, 2t+k] = sum_d wd[s(p,t), d] * e_k[d]
            psA = al_pool.tile([128, 2 * NT], F32, tag="psA")
            psAs[b] = psA
            for c in range(NCH):
                # First chunk of the kernel arrives in 1MB quarters so the
                # DVE starts ~6us earlier; steady state uses one 2MB DMA.
                nparts = 4 if (b == 0 and c == 0) else 1
                jpp = JP // nparts
                wd_ch = wd_pool.tile([128, JP * D], F32, tag="wd")
                full = chunk_view(wd_h, b, c)
                for p_ in range(nparts):
                    fsl = slice(p_ * jpp * D, (p_ + 1) * jpp * D)
                    nc.sync.dma_start(wd_ch[:, fsl], full[:, fsl])
                for j in range(JP):
                    t = c * JP + j
                    wsl = wd_ch[:, j * D:(j + 1) * D]
                    for k in range(2):
                        scr = scr_pool.tile([128, D], F32, tag="scr")
                        nc.vector.scalar_tensor_tensor(
                            scr[:], wsl, 1.0,
                            ebc[:, (b * 2 + k) * D:(b * 2 + k + 1) * D],
                            op0=ALU.mult, op1=ALU.mult,
                            accum_out=psA[:, 2 * t + k:2 * t + k + 1])

        def phase23(b):
            psA = psAs.pop(b)
            # ---- softmax ----
            mx = sm_pool.tile([128, 1], F32, tag="mx")
            nc.vector.tensor_reduce(mx[:], psA[:], axis=mybir.AxisListType.X,
                                    op=ALU.max)
            tmax = pss_pool.tile([1, 128], F32, tag="pssm")
            nc.tensor.transpose(tmax[:], mx[:], ident[:])
            m2 = sm_pool.tile([1, 1], F32, tag="m2")
            nc.vector.tensor_reduce(m2[:], tmax[:], axis=mybir.AxisListType.X,
                                    op=ALU.max)
            # broadcast -maxA to all partitions: [128,1] psum -> sbuf
            mneg_ps = pss_pool.tile([128, 1], F32, tag="pssm")
            nc.tensor.matmul(mneg_ps[:], negone[:], m2[:], start=True, stop=True)
            mneg = sm_pool.tile([128, 1], F32, tag="mneg")
            nc.scalar.copy(mneg[:], mneg_ps[:])

            # E_k = exp(A_k - maxA) with per-partition accumulate
            psA_v = psA[:].rearrange("p (t k) -> p k t", k=2)
            E = al_pool.tile([128, 2 * NT], F32, tag="E")
            E_v = E[:].rearrange("p (t k) -> p k t", k=2)
            s12 = sm_pool.tile([128, 2], F32, tag="s12")
            for k in range(2):
                nc.scalar.activation(E_v[:, k, :], psA_v[:, k, :], AF.Exp,
                                     bias=mneg[:], scale=1.0,
                                     accum_out=s12[:, k:k + 1])
            # cross-partition sum -> Z'[1,2]; c_k = 0.5 / Z'_k^2
            zsum = pss_pool.tile([1, 2], F32, tag="pssm")
            nc.tensor.matmul(zsum[:], onescol[:], s12[:], start=True, stop=True)
            zinv = sm_pool.tile([1, 2], F32, tag="zinv")
            nc.vector.reciprocal(zinv[:], zsum[:])
            zz = sm_pool.tile([1, 2], F32, tag="zz")
            nc.vector.tensor_scalar(zz[:], zinv[:], 0.5, None, op0=ALU.mult)
            nc.vector.tensor_mul(zz[:], zz[:], zinv[:])
            # broadcast c_k to all partitions: [128, 2] psum -> sbuf
            c_ps = pss_pool.tile([128, 2], F32, tag="pssm")
            nc.tensor.matmul(c_ps[:], onesrow[:], zz[:], start=True, stop=True)
            c12 = sm_pool.tile([128, 2], F32, tag="c12")
            nc.scalar.copy(c12[:], c_ps[:])

            # alpha = c_1*E_1^2 + c_2*E_2^2
            esq = al_pool.tile([128, 2 * NT], F32, tag="esq")
            nc.vector.tensor_mul(esq[:], E[:], E[:])
            esq_v = esq[:].rearrange("p (t k) -> p k t", k=2)
            atmp = al_pool.tile([128, NT], F32, tag="atmp")
            nc.vector.tensor_scalar_mul(atmp[:], esq_v[:, 1, :], c12[:, 1:2])
            alpha = al_pool.tile([128, NT], F32, tag="alpha")
            nc.vector.scalar_tensor_tensor(alpha[:], esq_v[:, 0, :],
                                           c12[:, 0:1], atmp[:],
                                           op0=ALU.mult, op1=ALU.add)

            # ---- out = wM * alpha ----
            # ACT is cheaper to spare than DVE late in the kernel: the last
            # batch leans on the DVE (194ns/tile vs ~660ns on ACT).
            # The last batch's wM lands last on the load ring and its stores
            # are the kernel tail: stream it in 1MB quarters so finals and
            # stores pipeline tightly behind the loads.
            nparts = 4 if b == BPC - 1 else 1
            jpp = JP // nparts
            for c in range(NCH):
                wm_ch = wm_pool.tile([128, JP * D], F32, tag="wm")
                out_ch = out_pool.tile([128, JP * D], F32, tag="out")
                wm_full = chunk_view(wM_h, b, c)
                out_full = chunk_view(out_h, b, c)
                for p_ in range(nparts):
                    fsl = slice(p_ * jpp * D, (p_ + 1) * jpp * D)
                    nc.sync.dma_start(wm_ch[:, fsl], wm_full[:, fsl])
                    for j in range(p_ * jpp, (p_ + 1) * jpp):
                        t = c * JP + j
                        sl = slice(j * D, (j + 1) * D)
                        if j % 2 == 0 or b == BPC - 1:
                            nc.vector.tensor_scalar_mul(out_ch[:, sl],
                                                        wm_ch[:, sl],
                                                        alpha[:, t:t + 1])
                        else:
                            nc.scalar.mul(out_ch[:, sl], wm_ch[:, sl],
                                          alpha[:, t:t + 1])
                    nc.scalar.dma_start(out_full[:, fsl], out_ch[:, fsl])

        # software pipeline: keep the DVE busy with batch b+1's dot products
        # while batch b's stats chain hops across engines.
        phase1(0)
        for b in range(BPC):
            if b + 1 < BPC:
                phase1(b + 1)
            phase23(b)

    nc.finalize()
    return nc


def _get_nc():
    if "nc" not in _cache:
        _cache["nc"] = _build()
    return _cache["nc"]


def _in_maps(wM, wd, e1, e2):
    ident = np.eye(128, dtype=np.float32)
    maps = []
    for i in range(N_CORES):
        sl = slice(i * BPC, (i + 1) * BPC)
        erow = np.ascontiguousarray(
            np.stack([e1[sl], e2[sl]], axis=1).reshape(1, BPC * 2 * D))
        maps.append({
            "wd": np.ascontiguousarray(wd[sl]),
            "wM": np.ascontiguousarray(wM[sl]),
            "erow": erow,
            "ident": ident,
        })
    return maps


def _run(wM, wd, e1, e2, **kw):
    wM = np.asarray(wM, dtype=np.float32)
    wd = np.asarray(wd, dtype=np.float32)
    e1 = np.asarray(e1, dtype=np.float32)
    e2 = np.asarray(e2, dtype=np.float32)
    nc = _get_nc()
    res = run_bass_kernel_spmd(nc, _in_maps(wM, wd, e1, e2), CORE_IDS, **kw)
    out = np.concatenate([r["out"] for r in res.results], axis=0)
    return out, res


def kernel(wM, wd, e1, e2):
    out, _ = _run(wM, wd, e1, e2)
    return out


# revision 26
# speedup vs baseline: 1.6097x; 1.0089x over previous
"""Trainium2 Bass kernel for entity-attention input scaling.

Computes, per batch row b:
    A_k = wd[b] @ e_k[b]          (k = 1, 2)   [S]
    alpha_k = softmax(A_k)
    out[b]  = wM[b] * 0.5 * (alpha_1^2 + alpha_2^2)[:, None]

Sharding: pure data parallel over the batch dim. B=32 batches are split
4-per-core over 8 NeuronCores; no cross-core communication.

Per-core pipeline (per local batch), memory-roofline bound (~50MB DMA/core):
  - wd streamed in 2MB contiguous chunks -> SBUF [128, 4096]
    (s = 2048*c + 16*p + j; p = partition, j in 0..15)
  - logits on the DVE: one fused scalar_tensor_tensor (product + free-axis
    accumulate) per [128, 256] tile against host-broadcast e_k
    -> psA[128, 64] (A_k per (tile, k) col).
  - softmax stats: row max via DVE reduce + PE transpose + DVE reduce;
    exp on ACT with per-partition accumulate, cross-partition sums via a
    ones-vector matmul, 1/Z on DVE reciprocal.  alpha is assembled as
    c_1*E_1^2 + c_2*E_2^2 with c_k = 0.5/Z_k^2 broadcast across partitions
    by a rank-1 matmul (no Ln -> single ACT table load).
  - out = wM * alpha via per-partition scaled multiply, split ACT/DVE.
  - The per-batch stats chain is a long cross-engine dependency chain, so
    emission is software-pipelined: phase 1 of batch b+1 is emitted before
    stats/finals of batch b, letting the DVE keep streaming dot products
    while batch b's stats hop across engines.
"""

import numpy as np
from contextlib import ExitStack

import concourse.bacc as bacc
import concourse.tile as tile
from concourse import mybir
from concourse.bass_utils import run_bass_kernel_spmd

B, S, D = 32, 4096, 256
N_CORES = 8
BPC = B // N_CORES          # batches per core
CHUNK = 2048                # S-rows per DMA chunk (2MB)
NCH = S // CHUNK            # chunks per batch
JP = CHUNK // 128           # 128-row tiles per chunk
NT = S // 128               # 128-row tiles per batch
F32 = mybir.dt.float32
AF = mybir.ActivationFunctionType
ALU = mybir.AluOpType
CORE_IDS = list(range(N_CORES))

_cache: dict = {}


def _build():
    nc = bacc.Bacc("TRN2", target_bir_lowering=False, debug=False,
                   num_devices=N_CORES)
    wd_h = nc.declare_dram_parameter("wd", [BPC, S, D], F32, isOutput=False)
    wM_h = nc.declare_dram_parameter("wM", [BPC, S, D], F32, isOutput=False)
    # erow[0, ((b*2 + k)*D + d)] = e_k[b, d]; broadcast on-chip (8KB DMA
    # instead of a 1MB pre-broadcast copy)
    er_h = nc.declare_dram_parameter("erow", [1, BPC * 2 * D], F32,
                                     isOutput=False)
    id_h = nc.declare_dram_parameter("ident", [128, 128], F32, isOutput=False)
    out_h = nc.declare_dram_parameter("out", [BPC, S, D], F32, isOutput=True)

    def chunk_view(h, b, c):
        # [CHUNK, D] contiguous rows -> [128, JP*D]; s = CHUNK*c + JP*p + j
        return h[b, CHUNK * c:CHUNK * (c + 1), :].rearrange(
            "(p j) d -> p (j d)", p=128)

    with tile.TileContext(nc) as tc, ExitStack() as ctx:
        consts = ctx.enter_context(tc.tile_pool(name="consts", bufs=1))
        wd_pool = ctx.enter_context(tc.tile_pool(name="wdp", bufs=3))
        wm_pool = ctx.enter_context(tc.tile_pool(name="wmp", bufs=4))
        out_pool = ctx.enter_context(tc.tile_pool(name="outp", bufs=3))
        scr_pool = ctx.enter_context(tc.tile_pool(name="scrp", bufs=2))
        sm_pool = ctx.enter_context(tc.tile_pool(name="smalls", bufs=2))
        al_pool = ctx.enter_context(tc.tile_pool(name="alphas", bufs=2))
        # two stats chains can be in flight at the kernel tail
        pss_pool = ctx.enter_context(tc.tile_pool(name="pss", bufs=4, space="PSUM"))
        psb_pool = ctx.enter_context(tc.tile_pool(name="psb", bufs=2, space="PSUM"))

        # constants: memset where possible, tiny DMAs on the store ring
        # (idle at kernel start) so nothing delays the first wd chunks.
        onescol = consts.tile([128, 1], F32)
        nc.gpsimd.memset(onescol[:], 1.0)
        onesrow = consts.tile([1, 128], F32)
        nc.gpsimd.memset(onesrow[:], 1.0)
        negone = consts.tile([1, 128], F32)
        nc.gpsimd.memset(negone[:], -1.0)
        ident = consts.tile([128, 128], F32)
        nc.scalar.dma_start(ident[:], id_h[:])
        # e rows: 8KB DMA, then rank-1 matmul broadcast to all partitions
        erow = consts.tile([1, BPC * 2 * D], F32)
        nc.scalar.dma_start(erow[:], er_h[:])
        ebc = consts.tile([128, BPC * 2 * D], F32)
        for q in range(BPC * 2 * D // 512):
            qsl = slice(q * 512, (q + 1) * 512)
            eb_ps = psb_pool.tile([128, 512], F32, tag="ebps")
            nc.tensor.matmul(eb_ps[:], onesrow[:], erow[:, qsl],
                             start=True, stop=True)
            nc.scalar.copy(ebc[:, qsl], eb_ps[:])

        psAs = {}

        def phase1(b, interleave=None):
            # logits: psA[p, 2t+k] = sum_d wd[s(p,t), d] * e_k[d]
            # `interleave`: list of closures (previous batch's stats/finals)
            # emitted one per j-tile so the DVE program keeps streaming dot
            # products while the stats chain hops across engines.
            psA = al_pool.tile([128, 2 * NT], F32, tag="psA")
            psAs[b] = psA
            for c in range(NCH):
                # First chunk of the kernel arrives in 1MB quarters so the
                # DVE starts ~6us earlier; steady state uses one 2MB DMA.
                nparts = 4 if (b == 0 and c == 0) else 1
                jpp = JP // nparts
                wd_ch = wd_pool.tile([128, JP * D], F32, tag="wd")
                full = chunk_view(wd_h, b, c)
                for p_ in range(nparts):
                    fsl = slice(p_ * jpp * D, (p_ + 1) * jpp * D)
                    nc.sync.dma_start(wd_ch[:, fsl], full[:, fsl])
                for j in range(JP):
                    t = c * JP + j
                    wsl = wd_ch[:, j * D:(j + 1) * D]
                    for k in range(2):
                        scr = scr_pool.tile([128, D], F32, tag="scr")
                        nc.vector.scalar_tensor_tensor(
                            scr[:], wsl, 1.0,
                            ebc[:, (b * 2 + k) * D:(b * 2 + k + 1) * D],
                            op0=ALU.mult, op1=ALU.mult,
                            accum_out=psA[:, 2 * t + k:2 * t + k + 1])
                    if interleave:
                        interleave.pop(0)()
            while interleave:
                interleave.pop(0)()

        def build_phase23_ops(b):
            """Batch b's softmax + finals as a list of closures, emitted one
            per j-tile inside the next batch's phase 1 (or directly)."""
            psA = psAs.pop(b)
            st: dict = {}
            ops = []

            def op_mx():
                st["mx"] = sm_pool.tile([128, 1], F32, tag="mx", name="mx")
                nc.vector.tensor_reduce(st["mx"][:], psA[:],
                                        axis=mybir.AxisListType.X, op=ALU.max)

            def op_tmax():
                st["tmax"] = pss_pool.tile([1, 128], F32, tag="pssm", name="tmax")
                nc.tensor.transpose(st["tmax"][:], st["mx"][:], ident[:])

            def op_m2():
                st["m2"] = sm_pool.tile([1, 1], F32, tag="m2", name="m2")
                nc.vector.tensor_reduce(st["m2"][:], st["tmax"][:],
                                        axis=mybir.AxisListType.X, op=ALU.max)

            def op_mneg_mm():
                st["mneg_ps"] = pss_pool.tile([128, 1], F32, tag="pssm", name="mneg_ps")
                nc.tensor.matmul(st["mneg_ps"][:], negone[:], st["m2"][:],
                                 start=True, stop=True)

            def op_mneg_cp():
                st["mneg"] = sm_pool.tile([128, 1], F32, tag="mneg", name="mneg")
                nc.scalar.copy(st["mneg"][:], st["mneg_ps"][:])

            def op_exp(k):
                psA_v = psA[:].rearrange("p (t k) -> p k t", k=2)
                if "E" not in st:
                    st["E"] = al_pool.tile([128, 2 * NT], F32, tag="E", name="E")
                    st["s12"] = sm_pool.tile([128, 2], F32, tag="s12", name="s12")
                E_v = st["E"][:].rearrange("p (t k) -> p k t", k=2)
                nc.scalar.activation(E_v[:, k, :], psA_v[:, k, :], AF.Exp,
                                     bias=st["mneg"][:], scale=1.0,
                                     accum_out=st["s12"][:, k:k + 1])

            def op_zsum():
                st["zsum"] = pss_pool.tile([1, 2], F32, tag="pssm", name="zsum")
                nc.tensor.matmul(st["zsum"][:], onescol[:], st["s12"][:],
                                 start=True, stop=True)

            def op_zinv():
                st["zinv"] = sm_pool.tile([1, 2], F32, tag="zinv", name="zinv")
                nc.vector.reciprocal(st["zinv"][:], st["zsum"][:])
                st["zz"] = sm_pool.tile([1, 2], F32, tag="zz", name="zz")
                nc.vector.tensor_scalar(st["zz"][:], st["zinv"][:], 0.5, None,
                                        op0=ALU.mult)
                nc.vector.tensor_mul(st["zz"][:], st["zz"][:], st["zinv"][:])

            def op_cps():
                st["c_ps"] = pss_pool.tile([128, 2], F32, tag="pssm", name="c_ps")
                nc.tensor.matmul(st["c_ps"][:], onesrow[:], st["zz"][:],
                                 start=True, stop=True)

            def op_c12():
                st["c12"] = sm_pool.tile([128, 2], F32, tag="c12", name="c12")
                nc.scalar.copy(st["c12"][:], st["c_ps"][:])

            def op_esq():
                st["esq"] = al_pool.tile([128, 2 * NT], F32, tag="esq", name="esq")
                nc.vector.tensor_mul(st["esq"][:], st["E"][:], st["E"][:])

            def op_alpha():
                esq_v = st["esq"][:].rearrange("p (t k) -> p k t", k=2)
                atmp = al_pool.tile([128, NT], F32, tag="atmp")
                nc.vector.tensor_scalar_mul(atmp[:], esq_v[:, 1, :],
                                            st["c12"][:, 1:2])
                st["alpha"] = al_pool.tile([128, NT], F32, tag="alpha", name="alpha")
                nc.vector.scalar_tensor_tensor(st["alpha"][:], esq_v[:, 0, :],
                                               st["c12"][:, 0:1], atmp[:],
                                               op0=ALU.mult, op1=ALU.add)

            ops += [op_mx, op_tmax, op_m2, op_mneg_mm, op_mneg_cp,
                    lambda: op_exp(0), lambda: op_exp(1),
                    op_zsum, op_zinv, op_cps, op_c12, op_esq, op_alpha]

            # ---- out = wM * alpha ----
            # Last batch: 1MB quarters + all finals on the DVE so loads,
            # finals and stores pipeline tightly at the kernel tail.
            nparts = 4 if b == BPC - 1 else 1
            jpp = JP // nparts
            for c in range(NCH):
                def op_wm_alloc(b=b, c=c):
                    st[("wm", c)] = wm_pool.tile([128, JP * D], F32, tag="wm", name="wm")
                    st[("out", c)] = out_pool.tile([128, JP * D], F32, tag="out", name="out")
                ops.append(op_wm_alloc)
                for p_ in range(nparts):
                    def op_part(b=b, c=c, p_=p_):
                        wm_ch = st[("wm", c)]
                        out_ch = st[("out", c)]
                        fsl = slice(p_ * jpp * D, (p_ + 1) * jpp * D)
                        nc.sync.dma_start(wm_ch[:, fsl],
                                          chunk_view(wM_h, b, c)[:, fsl])
                        for j in range(p_ * jpp, (p_ + 1) * jpp):
                            t = c * JP + j
                            sl = slice(j * D, (j + 1) * D)
                            if j % 2 == 0 or b == BPC - 1:
                                nc.vector.tensor_scalar_mul(
                                    out_ch[:, sl], wm_ch[:, sl],
                                    st["alpha"][:, t:t + 1])
                            else:
                                nc.scalar.mul(out_ch[:, sl], wm_ch[:, sl],
                                              st["alpha"][:, t:t + 1])
                        nc.scalar.dma_start(
                            chunk_view(out_h, b, c)[:, fsl], out_ch[:, fsl])
                    ops.append(op_part)
            return ops

        # software pipeline: batch b's stats/finals closures are emitted one
        # per j-tile inside batch b+2's phase 1 so the DVE keeps streaming
        # dot products while the stats chain hops across engines.  The two
        # trailing batches' chains are interleaved with each other.
        phase1(0)
        phase1(1)
        phase1(2, interleave=build_phase23_ops(0))
        phase1(3, interleave=build_phase23_ops(1))
        ops2 = build_phase23_ops(2)
        ops3 = build_phase23_ops(3)
        while ops2 or ops3:
            if ops2:
                ops2.pop(0)()
            if ops3:
                ops3.pop(0)()

    nc.finalize()
    return nc


def _get_nc():
    if "nc" not in _cache:
        _cache["nc"] = _build()
    return _cache["nc"]


def _in_maps(wM, wd, e1, e2):
    ident = np.eye(128, dtype=np.float32)
    maps = []
    for i in range(N_CORES):
        sl = slice(i * BPC, (i + 1) * BPC)
        erow = np.ascontiguousarray(
            np.stack([e1[sl], e2[sl]], axis=1).reshape(1, BPC * 2 * D))
        maps.append({
            "wd": np.ascontiguousarray(wd[sl]),
            "wM": np.ascontiguousarray(wM[sl]),
            "erow": erow,
            "ident": ident,
        })
    return maps


def _run(wM, wd, e1, e2, **kw):
    wM = np.asarray(wM, dtype=np.float32)
    wd = np.asarray(wd, dtype=np.float32)
    e1 = np.asarray(e1, dtype=np.float32)
    e2 = np.asarray(e2, dtype=np.float32)
    nc = _get_nc()
    res = run_bass_kernel_spmd(nc, _in_maps(wM, wd, e1, e2), CORE_IDS, **kw)
    out = np.concatenate([r["out"] for r in res.results], axis=0)
    return out, res


def kernel(wM, wd, e1, e2):
    out, _ = _run(wM, wd, e1, e2)
    return out
